# revision 7
# baseline (speedup 1.0000x reference)
"""Trainium2 Bass kernel for nn_Attention_81956565942967.

Cross-attention with key-length masking:
  B=8, N=1024, DIM=1024, HEADS=16, DIM_HEAD=64.

Sharding: pure data parallel — batch element b -> NeuronCore b. No
collectives. Host-side prep per shard: transpose x/context (so the
contraction dim lands on SBUF partitions) and cast the big operands to
bf16; compute the key-mask bias from lengths. bout is added host-side
(it is an epilogue broadcast add; exact in fp32).

Device algorithm (per core, T-layout: features on partitions):
  qT = Wq^T xT          kT = Wk^T cT          v = (cT)^T Wv   (natural)
  per head-pair hp, query tile i, per KEY TILE j:
    dotsT[j,i] two matmuls (sub=0/1 -> PE row groups 0/64, concurrent)
      into a [128, 2, 512] PSUM unit (double-buffered pool)
    one Exp ACT call per unit -> etall[:, j, :, :]   (bf16)
  per (hp, sub): po = [v_h | mask-diag]^T et_h  (K=128, M=80; psum rows
    64:80 carry the masked rowsum for head h at row 64+h)
  norm: reciprocal_approx_fast of the 16 rowsums, cast to bf16, K=16
    selector matmul broadcasts 1/rs across partitions; catT *= bcast.
  out = catT^T Wout  (bf16 output; bout added host-side)
Scheduling: qT/kT projections and early output-projection tiles are
interleaved into the attention loop as PE gap-filler; dots units are
double-buffered against the Exp stream so PE never waits on ACT.
"""

from contextlib import ExitStack

import ml_dtypes
import numpy as np

import concourse.bass as bass
from concourse import bacc
import concourse.mybir as mybir
import concourse.tile as tile
from concourse.bass_utils import run_bass_kernel_spmd

B, N, DIM = 8, 1024, 1024
HEADS, DIM_HEAD = 16, 64
INNER = HEADS * DIM_HEAD
SCALE = DIM_HEAD ** -0.5

P = 128
NT = N // P      # 8 partition tiles along n/j
KT = DIM // P    # 8 contraction tiles along dim/inner
FI = 512         # free-dim tile (PSUM bank)
NI = N // FI     # 2 query tiles
VW = 80  # v block per head: 64 dims + mask col at 64+h -> rowsum lands on psum row 64+h

BF = mybir.dt.bfloat16
F32 = mybir.dt.float32

_CACHE: dict = {}


def _build() -> bass.Bass:
    nc = bacc.Bacc("TRN2")

    xT_d = nc.dram_tensor("xT", [DIM, N], BF, kind="ExternalInput").ap()
    cT_d = nc.dram_tensor("cT", [DIM, N], BF, kind="ExternalInput").ap()
    wq_d = nc.dram_tensor("Wq", [DIM, INNER], BF, kind="ExternalInput").ap()
    wkv_d = nc.dram_tensor("Wkv", [DIM, 2 * INNER], BF, kind="ExternalInput").ap()
    wout_d = nc.dram_tensor("Wout", [INNER, DIM], BF, kind="ExternalInput").ap()
    mask_d = nc.dram_tensor("maskb", [P, NT], F32, kind="ExternalInput").ap()
    sel_d = nc.dram_tensor("sel", [80, (HEADS // 2) * P], BF, kind="ExternalInput").ap()
    out_d = nc.dram_tensor("out", [N, DIM], BF, kind="ExternalOutput").ap()

    with tile.TileContext(nc) as tc, ExitStack() as ctx:
        const_p = ctx.enter_context(tc.tile_pool(name="const", bufs=1))
        et_p = ctx.enter_context(tc.tile_pool(name="etp", bufs=2))
        stage_p = ctx.enter_context(tc.tile_pool(name="stage", bufs=3))
        acc_ps = ctx.enter_context(tc.tile_pool(name="accps", bufs=2, space="PSUM"))
        dots_ps = ctx.enter_context(tc.tile_pool(name="dotsps", bufs=2, space="PSUM"))
        po_ps = ctx.enter_context(tc.tile_pool(name="pops", bufs=2, space="PSUM"))

        # --- static SBUF tensors (declare before DMAs so order is explicit) ---
        xT_t = [const_p.tile([P, N], BF, tag=f"xslot{t}", name=f"xs{t}") for t in range(KT)]
        cT_t = [const_p.tile([P, N], BF, tag=f"cslot{t}", name=f"cs{t}") for t in range(KT)]
        wq_t = [const_p.tile([P, INNER], BF, tag=f"wq{t}", name=f"wq{t}") for t in range(KT)]
        wkv_t = [const_p.tile([P, 2 * INNER], BF, tag=f"wkv{t}", name=f"wkv{t}") for t in range(KT)]
        wout_t = [const_p.tile([P, DIM], BF, tag=f"cslot{t}", name=f"wo{t}") for t in range(KT)]
        qT_t = [const_p.tile([P, N], BF, tag=f"q{t}", name=f"qt{t}") for t in range(KT)]
        kT_t = [const_p.tile([P, N], BF, tag=f"k{t}", name=f"kt{t}") for t in range(KT)]
        v_t = [const_p.tile([P, HEADS * VW], BF, tag=f"v{t}", name=f"vt{t}") for t in range(NT)]
        catT_t = [const_p.tile([P, N], BF, tag=f"cat{t}", name=f"cat{t}") for t in range(KT)]

        mask_sb = const_p.tile([P, NT], F32, tag="mask")  # 1.0 valid / 0.0 masked
        sel_sb = const_p.tile([80, (HEADS // 2) * P], BF, tag="sel")
        rs_all = const_p.tile([80, N], F32, tag="rsall")
        rr_all = const_p.tile([80, N], F32, tag="rrall")
        rr_bf = const_p.tile([80, N], BF, tag="rrbf")

        # --- input DMAs, in consumption order. The dots->exp stream needs
        # xT (all), wq[0], cT (all), and the K-halves of wkv first; wkv
        # V-halves and the remaining wq tiles follow. First tiles striped
        # across queues for low latency. ---
        SPLIT = 4
        for s in range(SPLIT):
            sl = slice(s * (INNER // SPLIT), (s + 1) * (INNER // SPLIT))
            nc.sync.dma_start(out=wq_t[0][:, sl], in_=wq_d[0:P, sl])
        for s in range(SPLIT):
            sl = slice(s * (N // SPLIT), (s + 1) * (N // SPLIT))
            nc.sync.dma_start(out=xT_t[0][:, sl], in_=xT_d[0:P, sl])
        for t in range(1, KT):
            nc.sync.dma_start(out=xT_t[t], in_=xT_d[t * P:(t + 1) * P, :])
        for t in range(KT):  # just the wkv columns proj_k(0) contracts
            nc.sync.dma_start(out=wkv_t[t][:, 0:P],
                              in_=wkv_d[t * P:(t + 1) * P, 0:P])
        for t in range(KT):
            nc.sync.dma_start(out=cT_t[t], in_=cT_d[t * P:(t + 1) * P, :])
        nc.sync.dma_start(out=mask_sb, in_=mask_d)
        for t in range(KT):
            nc.sync.dma_start(out=wkv_t[t][:, P:INNER],
                              in_=wkv_d[t * P:(t + 1) * P, P:INNER])
            nc.sync.dma_start(out=wkv_t[t][:, INNER:2 * INNER],
                              in_=wkv_d[t * P:(t + 1) * P, INNER:2 * INNER])
            if t > 0:
                nc.sync.dma_start(out=wq_t[t], in_=wq_d[t * P:(t + 1) * P, :])
        nc.sync.dma_start(out=sel_sb, in_=sel_d)
        for t in range(NT):
            nc.vector.memset(v_t[t], 0.0)
        nc.vector.memset(rs_all[64:80, :], 0.0)

        # --- projection helpers ---
        def proj_q(m, only_i=None):
            for i in range(NI) if only_i is None else [only_i]:
                ps = acc_ps.tile([P, FI], F32, tag="acc", name="ps")
                for k in range(KT):
                    nc.tensor.matmul(
                        ps,
                        wq_t[k][:, m * P:(m + 1) * P],
                        xT_t[k][:, i * FI:(i + 1) * FI],
                        start=(k == 0), stop=(k == KT - 1),
                    )
                nc.vector.tensor_copy(qT_t[m][:, i * FI:(i + 1) * FI], ps)

        def proj_k(m):
            for i in range(NI):
                ps = acc_ps.tile([P, FI], F32, tag="acc", name="ps")
                for k in range(KT):
                    nc.tensor.matmul(
                        ps,
                        wkv_t[k][:, m * P:(m + 1) * P],
                        cT_t[k][:, i * FI:(i + 1) * FI],
                        start=(k == 0), stop=(k == KT - 1),
                    )
                nc.vector.tensor_copy(kT_t[m][:, i * FI:(i + 1) * FI], ps)

        def proj_v(t):
            v3 = v_t[t].rearrange("p (h w) -> p h w", w=VW)
            for i2 in range(NI):
                ps = acc_ps.tile([P, FI], F32, tag="acc", name="ps")
                for k in range(KT):
                    nc.tensor.matmul(
                        ps,
                        cT_t[k][:, t * P:(t + 1) * P],
                        wkv_t[k][:, INNER + i2 * FI:INNER + (i2 + 1) * FI],
                        start=(k == 0), stop=(k == KT - 1),
                    )
                nc.vector.tensor_copy(
                    v3[:, i2 * 8:(i2 + 1) * 8, 0:DIM_HEAD],
                    ps.rearrange("p (h d) -> p h d", d=DIM_HEAD),
                )
            # mask col of head h sits at flat position h*VW + 64 + h = 64 + 81*h
            diag = bass.AP(tensor=v_t[t].tensor, offset=v_t[t].offset + DIM_HEAD,
                           ap=[list(v_t[t].ap[0]), [VW + 1, HEADS]])
            nc.vector.tensor_scalar_mul(
                diag, mask_sb[:, t:t + 1].to_broadcast([P, HEADS]), 1.0)

        def final_group(t, i, tail=False):
            pf = acc_ps.tile([P, FI], F32, tag="acc", name="pf")
            for k in range(KT):
                nc.tensor.matmul(
                    pf,
                    catT_t[k][:, t * P:(t + 1) * P],
                    wout_t[k][:, i * FI:(i + 1) * FI],
                    start=(k == 0), stop=(k == KT - 1),
                )
            ot = stage_p.tile([P, FI], BF, tag="ot", name="ot")
            if tail:
                nc.scalar.copy(ot, pf)  # ACT engine is idle at the kernel tail
            else:
                nc.vector.tensor_copy(ot, pf)
            dst = out_d[t * P:(t + 1) * P, i * FI:(i + 1) * FI]
            if tail:
                # stripe the tail output DMAs across queues to shrink drain
                for s in range(2):
                    nc.sync.dma_start(
                        out=dst[:, s * (FI // 2):(s + 1) * (FI // 2)],
                        in_=ot[:, s * (FI // 2):(s + 1) * (FI // 2)],
                    )
            else:
                nc.sync.dma_start(out=dst, in_=ot)

        def attention(i, hp):
            isl = slice(i * FI, (i + 1) * FI)
            pt = hp
            et = et_p.tile([P, NT, 2, FI], BF, tag="et", name="et")
            # per key-tile unit: paired dots (heads 2hp/2hp+1 at PE row
            # groups 0/64 -> concurrent) + one Exp call; double-buffered
            for j in range(NT):
                dps = dots_ps.tile([P, 2, FI], F32, tag="dots", name="dps")
                for sub in range(2):
                    off = sub * DIM_HEAD
                    nc.tensor.matmul(
                        dps[:, sub, :],
                        kT_t[pt][off:off + DIM_HEAD, j * P:(j + 1) * P],
                        qT_t[pt][off:off + DIM_HEAD, isl],
                        start=True, stop=True,
                    )
                nc.scalar.activation(
                    et[:, j, :, :], dps,
                    mybir.ActivationFunctionType.Exp, scale=SCALE,
                )
            for sub in range(2):
                h = 2 * hp + sub
                off = sub * DIM_HEAD
                po = po_ps.tile([VW, FI], F32, tag="po", name="po")
                for j in range(NT):
                    nc.tensor.matmul(
                        po,
                        v_t[j][:, h * VW:(h + 1) * VW],
                        et[:, j, sub, :],
                        start=(j == 0), stop=(j == NT - 1),
                    )
                cslice = catT_t[pt][off:off + DIM_HEAD, isl]
                nc.vector.tensor_copy(cslice, po[0:DIM_HEAD, :])
                nc.vector.tensor_tensor(
                    rs_all[64:80, isl], rs_all[64:80, isl], po[64:80, :],
                    mybir.AluOpType.add)

        def norm(i):
            isl = slice(i * FI, (i + 1) * FI)
            nc.vector.reciprocal(rr_all[64:80, isl], rs_all[64:80, isl])
            nc.vector.tensor_copy(rr_bf[64:80, isl], rr_all[64:80, isl])
            for pt in range(HEADS // 2):
                pb = acc_ps.tile([P, FI], F32, tag="acc", name="pb")
                nc.tensor.matmul(
                    pb,
                    sel_sb[64:80, pt * P:(pt + 1) * P],
                    rr_bf[64:80, isl],
                    start=True, stop=True,
                )
                nc.vector.tensor_tensor(
                    catT_t[pt][:, isl], catT_t[pt][:, isl], pb,
                    mybir.AluOpType.mult,
                )

        # --- schedule: start the dots->exp stream as early as possible;
        # projections become PE gap-filler inside the attention loop.
        # qT's i=1 halves are deferred into phase 1 to balance PE load. ---
        proj_q(0)
        proj_k(0)
        proj_q(1, only_i=0)
        proj_k(1)
        for t in range(NT):
            proj_v(t)
        for hp in range(HEADS // 2):
            attention(0, hp)
            if hp + 2 < KT:
                proj_q(hp + 2, only_i=0)
                proj_k(hp + 2)
        for t in range(KT):
            nc.sync.dma_start(out=wout_t[t], in_=wout_d[t * P:(t + 1) * P, :])

        # keep the ACT stream rolling into phase 1 before norm(0)
        attention(1, 0)
        proj_q(1, only_i=1)
        norm(0)
        fin = [(t, i) for t in range(4) for i in range(NI)]
        for hp in range(1, HEADS // 2):
            attention(1, hp)
            if hp + 1 < KT:
                proj_q(hp + 1, only_i=1)
            final_group(*fin[hp - 1])
        for t, i in fin[7:]:
            final_group(t, i)
        norm(1)
        for t in range(4, NT):
            for i in range(NI):
                final_group(t, i, tail=True)

    nc.finalize()
    return nc


def _prep_in_maps(x, context, lengths, Wq, Wkv, Wout, bout):
    bf = ml_dtypes.bfloat16
    wq = np.ascontiguousarray(Wq, dtype=bf)
    wkv = np.ascontiguousarray(Wkv, dtype=bf)
    wout = np.ascontiguousarray(Wout, dtype=bf)
    jj = np.arange(N).reshape(NT, P)  # [j_tile, partition]
    sel = np.zeros((80, (HEADS // 2) * P), dtype=bf)
    for pt in range(HEADS // 2):
        sel[64 + 2 * pt, pt * P:pt * P + DIM_HEAD] = 1.0
        sel[65 + 2 * pt, pt * P + DIM_HEAD:(pt + 1) * P] = 1.0
    in_maps = []
    context = np.asarray(context)
    for b in range(B):
        mb = np.where(jj < int(lengths[b]), 1.0, 0.0).astype(np.float32)
        cb = context[b].copy()
        cb[int(lengths[b]):] = 0.0
        in_maps.append({
            "xT": np.ascontiguousarray(np.asarray(x[b]).T, dtype=bf),
            "cT": np.ascontiguousarray(cb.T, dtype=bf),
            "Wq": wq, "Wkv": wkv, "Wout": wout,
            "maskb": np.ascontiguousarray(mb.T), "sel": sel,
        })
    return in_maps


def run(inputs: dict, trace: bool = False):
    if "nc" not in _CACHE:
        _CACHE["nc"] = _build()
    nc = _CACHE["nc"]
    in_maps = _prep_in_maps(**inputs)
    res = run_bass_kernel_spmd(nc, in_maps, core_ids=list(range(B)), trace=trace)
    out = np.stack([res.results[i]["out"] for i in range(B)]).astype(np.float32)
    out += np.asarray(inputs["bout"], dtype=np.float32)[None, None, :]
    return out, res


def kernel(**inputs) -> np.ndarray:
    out, _ = run(inputs, trace=False)
    return out


# revision 12
# speedup vs baseline: 1.1553x; 1.1553x over previous
"""Trainium2 Bass kernel for nn_Attention_81956565942967.

Cross-attention with key-length masking:
  B=8, N=1024, DIM=1024, HEADS=16, DIM_HEAD=64.

Sharding: pure data parallel — batch element b -> NeuronCore b. No
collectives. Host-side prep per shard: transpose x/context (so the
contraction dim lands on SBUF partitions) and cast the big operands to
bf16; compute the key-mask bias from lengths. bout is added host-side
(it is an epilogue broadcast add; exact in fp32).

Device algorithm (per core, T-layout: features on partitions):
  qT = Wq^T xT          kT = Wk^T cT          v = (cT)^T Wv   (natural)
  per head-pair hp, query tile i, per KEY TILE j:
    dotsT[j,i] two matmuls (sub=0/1 -> PE row groups 0/64, concurrent)
      into a [128, 2, 512] PSUM unit (double-buffered pool)
    one Exp ACT call per unit -> etall[:, j, :, :]   (bf16)
  per (hp, sub): po = [v_h | mask-diag]^T et_h  (K=128, M=80; psum rows
    64:80 carry the masked rowsum for head h at row 64+h)
  norm: reciprocal_approx_fast of the 16 rowsums, cast to bf16, K=16
    selector matmul broadcasts 1/rs across partitions; catT *= bcast.
  out = catT^T Wout  (bf16 output; bout added host-side)
Scheduling: qT/kT projections and early output-projection tiles are
interleaved into the attention loop as PE gap-filler; dots units are
double-buffered against the Exp stream so PE never waits on ACT.
"""

from contextlib import ExitStack

import ml_dtypes
import numpy as np

import concourse.bass as bass
from concourse import bacc
import concourse.mybir as mybir
import concourse.tile as tile
from concourse.bass_utils import run_bass_kernel_spmd

B, N, DIM = 8, 1024, 1024
HEADS, DIM_HEAD = 16, 64
INNER = HEADS * DIM_HEAD
SCALE = DIM_HEAD ** -0.5

P = 128
NT = N // P      # 8 partition tiles along n/j
KT = DIM // P    # 8 contraction tiles along dim/inner
FI = 512         # free-dim tile (PSUM bank)
NI = N // FI     # 2 query tiles
VW = 80  # v block per head: 64 dims + mask col at 64+h -> rowsum lands on psum row 64+h

BF = mybir.dt.bfloat16
F32 = mybir.dt.float32

_CACHE: dict = {}


def _build() -> bass.Bass:
    nc = bacc.Bacc("TRN2")

    xT_d = nc.dram_tensor("xT", [DIM, N], BF, kind="ExternalInput").ap()
    cT_d = nc.dram_tensor("cT", [DIM, N], BF, kind="ExternalInput").ap()
    wq_d = nc.dram_tensor("Wq", [DIM, INNER], BF, kind="ExternalInput").ap()
    wkv_d = nc.dram_tensor("Wkv", [DIM, 2 * INNER], BF, kind="ExternalInput").ap()
    wout_d = nc.dram_tensor("Wout", [INNER, DIM], BF, kind="ExternalInput").ap()
    mask_d = nc.dram_tensor("maskb", [P, NT], F32, kind="ExternalInput").ap()
    sel_d = nc.dram_tensor("sel", [80, (HEADS // 2) * P], BF, kind="ExternalInput").ap()
    out_d = nc.dram_tensor("out", [N, DIM], BF, kind="ExternalOutput").ap()

    with tile.TileContext(nc) as tc, ExitStack() as ctx:
        const_p = ctx.enter_context(tc.tile_pool(name="const", bufs=1))
        et_p = ctx.enter_context(tc.tile_pool(name="etp", bufs=2))
        stage_p = ctx.enter_context(tc.tile_pool(name="stage", bufs=3))
        acc_ps = ctx.enter_context(tc.tile_pool(name="accps", bufs=2, space="PSUM"))
        dots_ps = ctx.enter_context(tc.tile_pool(name="dotsps", bufs=2, space="PSUM"))
        po_ps = ctx.enter_context(tc.tile_pool(name="pops", bufs=2, space="PSUM"))

        # --- static SBUF tensors (declare before DMAs so order is explicit) ---
        xT_t = [const_p.tile([P, N], BF, tag=f"xslot{t}", name=f"xs{t}") for t in range(KT)]
        cT_t = [const_p.tile([P, N], BF, tag=f"cslot{t}", name=f"cs{t}") for t in range(KT)]
        wq_t = [const_p.tile([P, INNER], BF, tag=f"wq{t}", name=f"wq{t}") for t in range(KT)]
        wkv_t = [const_p.tile([P, 2 * INNER], BF, tag=f"wkv{t}", name=f"wkv{t}") for t in range(KT)]
        wout_t = [const_p.tile([P, DIM], BF, tag=f"cslot{t}", name=f"wo{t}") for t in range(KT)]
        qT_t = [const_p.tile([P, N], BF, tag=f"q{t}", name=f"qt{t}") for t in range(KT)]
        kT_t = [const_p.tile([P, N], BF, tag=f"k{t}", name=f"kt{t}") for t in range(KT)]
        v_t = [const_p.tile([P, HEADS * VW], BF, tag=f"v{t}", name=f"vt{t}") for t in range(NT)]
        catT_t = [const_p.tile([P, N], BF, tag=f"cat{t}", name=f"cat{t}") for t in range(KT)]

        mask_sb = const_p.tile([P, NT], F32, tag="mask")  # 1.0 valid / 0.0 masked
        sel_sb = const_p.tile([80, (HEADS // 2) * P], BF, tag="sel")
        rs_all = const_p.tile([80, N], F32, tag="rsall")
        rr_all = const_p.tile([80, N], F32, tag="rrall")
        rr_bf = const_p.tile([80, N], BF, tag="rrbf")

        # --- input DMAs, in consumption order. The dots->exp stream needs
        # xT (all), wq[0], cT (all), and the K-halves of wkv first; wkv
        # V-halves and the remaining wq tiles follow. First tiles striped
        # across queues for low latency. ---
        SPLIT = 4
        for s in range(SPLIT):
            sl = slice(s * (INNER // SPLIT), (s + 1) * (INNER // SPLIT))
            nc.sync.dma_start(out=wq_t[0][:, sl], in_=wq_d[0:P, sl])
        for s in range(SPLIT):
            sl = slice(s * (N // SPLIT), (s + 1) * (N // SPLIT))
            nc.sync.dma_start(out=xT_t[0][:, sl], in_=xT_d[0:P, sl])
        for t in range(1, KT):
            nc.sync.dma_start(out=xT_t[t], in_=xT_d[t * P:(t + 1) * P, :])
        for t in range(KT):  # just the wkv columns proj_k(0) contracts
            nc.sync.dma_start(out=wkv_t[t][:, 0:P],
                              in_=wkv_d[t * P:(t + 1) * P, 0:P])
        for t in range(KT):
            nc.sync.dma_start(out=cT_t[t], in_=cT_d[t * P:(t + 1) * P, :])
        nc.sync.dma_start(out=mask_sb, in_=mask_d)
        for t in range(KT):
            nc.sync.dma_start(out=wkv_t[t][:, P:INNER],
                              in_=wkv_d[t * P:(t + 1) * P, P:INNER])
            nc.sync.dma_start(out=wkv_t[t][:, INNER:2 * INNER],
                              in_=wkv_d[t * P:(t + 1) * P, INNER:2 * INNER])
            if t > 0:
                nc.sync.dma_start(out=wq_t[t], in_=wq_d[t * P:(t + 1) * P, :])
        nc.sync.dma_start(out=sel_sb, in_=sel_d)
        for t in range(NT):
            nc.vector.memset(v_t[t], 0.0)
        nc.vector.memset(rs_all[64:80, :], 0.0)

        # --- projection helpers ---
        def proj_q(m, only_i=None):
            for i in range(NI) if only_i is None else [only_i]:
                ps = acc_ps.tile([P, FI], F32, tag="acc", name="ps")
                for k in range(KT):
                    nc.tensor.matmul(
                        ps,
                        wq_t[k][:, m * P:(m + 1) * P],
                        xT_t[k][:, i * FI:(i + 1) * FI],
                        start=(k == 0), stop=(k == KT - 1),
                    )
                nc.vector.tensor_copy(qT_t[m][:, i * FI:(i + 1) * FI], ps)

        def proj_k(m):
            for i in range(NI):
                ps = acc_ps.tile([P, FI], F32, tag="acc", name="ps")
                for k in range(KT):
                    nc.tensor.matmul(
                        ps,
                        wkv_t[k][:, m * P:(m + 1) * P],
                        cT_t[k][:, i * FI:(i + 1) * FI],
                        start=(k == 0), stop=(k == KT - 1),
                    )
                nc.vector.tensor_copy(kT_t[m][:, i * FI:(i + 1) * FI], ps)

        def proj_v(t):
            v3 = v_t[t].rearrange("p (h w) -> p h w", w=VW)
            for i2 in range(NI):
                ps = acc_ps.tile([P, FI], F32, tag="acc", name="ps")
                for k in range(KT):
                    nc.tensor.matmul(
                        ps,
                        cT_t[k][:, t * P:(t + 1) * P],
                        wkv_t[k][:, INNER + i2 * FI:INNER + (i2 + 1) * FI],
                        start=(k == 0), stop=(k == KT - 1),
                    )
                nc.vector.tensor_copy(
                    v3[:, i2 * 8:(i2 + 1) * 8, 0:DIM_HEAD],
                    ps.rearrange("p (h d) -> p h d", d=DIM_HEAD),
                )
            # mask col of head h sits at flat position h*VW + 64 + h = 64 + 81*h
            diag = bass.AP(tensor=v_t[t].tensor, offset=v_t[t].offset + DIM_HEAD,
                           ap=[list(v_t[t].ap[0]), [VW + 1, HEADS]])
            nc.vector.tensor_scalar_mul(
                diag, mask_sb[:, t:t + 1].to_broadcast([P, HEADS]), 1.0)

        def final_group(t, i, tail=False):
            pf = acc_ps.tile([P, FI], F32, tag="acc", name="pf")
            for k in range(KT):
                nc.tensor.matmul(
                    pf,
                    catT_t[k][:, t * P:(t + 1) * P],
                    wout_t[k][:, i * FI:(i + 1) * FI],
                    start=(k == 0), stop=(k == KT - 1),
                )
            ot = stage_p.tile([P, FI], BF, tag="ot", name="ot")
            if tail:
                nc.scalar.copy(ot, pf)  # ACT engine is idle at the kernel tail
            else:
                nc.vector.tensor_copy(ot, pf)
            dst = out_d[t * P:(t + 1) * P, i * FI:(i + 1) * FI]
            if tail:
                # stripe the tail output DMAs across queues to shrink drain
                for s in range(2):
                    nc.sync.dma_start(
                        out=dst[:, s * (FI // 2):(s + 1) * (FI // 2)],
                        in_=ot[:, s * (FI // 2):(s + 1) * (FI // 2)],
                    )
            else:
                nc.sync.dma_start(out=dst, in_=ot)

        def dots_exp(i, hp):
            isl = slice(i * FI, (i + 1) * FI)
            pt = hp
            et = et_p.tile([P, NT, 2, FI], BF, tag="et", name="et")
            # per key-tile unit: paired dots (heads 2hp/2hp+1 at PE row
            # groups 0/64 -> concurrent) + one Exp call; double-buffered
            for j in range(NT):
                dps = dots_ps.tile([P, 2, FI], F32, tag="dots", name="dps")
                for sub in range(2):
                    off = sub * DIM_HEAD
                    nc.tensor.matmul(
                        dps[:, sub, :],
                        kT_t[pt][off:off + DIM_HEAD, j * P:(j + 1) * P],
                        qT_t[pt][off:off + DIM_HEAD, isl],
                        start=True, stop=True,
                    )
                nc.scalar.activation(
                    et[:, j, :, :], dps,
                    mybir.ActivationFunctionType.Exp, scale=SCALE,
                )
            return et

        def po_part(i, hp, et):
            # NOTE: must be emitted after proj_v — the po read of the v
            # mask-diag (strided AP) is not reliably dependency-tracked.
            isl = slice(i * FI, (i + 1) * FI)
            pt = hp
            for sub in range(2):
                h = 2 * hp + sub
                off = sub * DIM_HEAD
                po = po_ps.tile([VW, FI], F32, tag="po", name="po")
                for j in range(NT):
                    nc.tensor.matmul(
                        po,
                        v_t[j][:, h * VW:(h + 1) * VW],
                        et[:, j, sub, :],
                        start=(j == 0), stop=(j == NT - 1),
                    )
                cslice = catT_t[pt][off:off + DIM_HEAD, isl]
                nc.vector.tensor_copy(cslice, po[0:DIM_HEAD, :])
                nc.vector.tensor_tensor(
                    rs_all[64:80, isl], rs_all[64:80, isl], po[64:80, :],
                    mybir.AluOpType.add)

        def attention(i, hp):
            et = dots_exp(i, hp)
            po_part(i, hp, et)

        def norm(i):
            isl = slice(i * FI, (i + 1) * FI)
            nc.vector.reciprocal(rr_all[64:80, isl], rs_all[64:80, isl])
            nc.vector.tensor_copy(rr_bf[64:80, isl], rr_all[64:80, isl])
            for pt in range(HEADS // 2):
                pb = acc_ps.tile([P, FI], F32, tag="acc", name="pb")
                nc.tensor.matmul(
                    pb,
                    sel_sb[64:80, pt * P:(pt + 1) * P],
                    rr_bf[64:80, isl],
                    start=True, stop=True,
                )
                nc.vector.tensor_tensor(
                    catT_t[pt][:, isl], catT_t[pt][:, isl], pb,
                    mybir.AluOpType.mult,
                )

        # --- schedule: start the dots->exp stream as early as possible;
        # projections become PE gap-filler inside the attention loop.
        # qT's i=1 halves are deferred into phase 1 to balance PE load. ---
        proj_q(0)
        proj_k(0)
        proj_q(1, only_i=0)
        proj_k(1)
        # attention(0,0)'s dots/exp go first; its po is emitted after
        # proj_v (the v mask-diag dep needs the emission order).
        et00 = dots_exp(0, 0)
        for t in range(NT):
            proj_v(t)
        po_part(0, 0, et00)
        proj_q(2, only_i=0)
        proj_k(2)
        for hp in range(1, HEADS // 2):
            attention(0, hp)
            if hp + 2 < KT:
                proj_q(hp + 2, only_i=0)
                proj_k(hp + 2)
        for t in range(KT):
            nc.sync.dma_start(out=wout_t[t], in_=wout_d[t * P:(t + 1) * P, :])

        # keep the ACT stream rolling into phase 1 before norm(0)
        attention(1, 0)
        proj_q(1, only_i=1)
        norm(0)
        fin = [(t, i) for t in range(4) for i in range(NI)]
        for hp in range(1, HEADS // 2):
            attention(1, hp)
            if hp + 1 < KT:
                proj_q(hp + 1, only_i=1)
            final_group(*fin[hp - 1])
        for t, i in fin[7:]:
            final_group(t, i)
        norm(1)
        for t in range(4, NT):
            for i in range(NI):
                final_group(t, i, tail=True)

    nc.finalize()
    return nc


def _prep_in_maps(x, context, lengths, Wq, Wkv, Wout, bout):
    bf = ml_dtypes.bfloat16
    wq = np.ascontiguousarray(Wq, dtype=bf)
    wkv = np.ascontiguousarray(Wkv, dtype=bf)
    wout = np.ascontiguousarray(Wout, dtype=bf)
    jj = np.arange(N).reshape(NT, P)  # [j_tile, partition]
    sel = np.zeros((80, (HEADS // 2) * P), dtype=bf)
    for pt in range(HEADS // 2):
        sel[64 + 2 * pt, pt * P:pt * P + DIM_HEAD] = 1.0
        sel[65 + 2 * pt, pt * P + DIM_HEAD:(pt + 1) * P] = 1.0
    in_maps = []
    context = np.asarray(context)
    for b in range(B):
        mb = np.where(jj < int(lengths[b]), 1.0, 0.0).astype(np.float32)
        cb = context[b].copy()
        cb[int(lengths[b]):] = 0.0
        in_maps.append({
            "xT": np.ascontiguousarray(np.asarray(x[b]).T, dtype=bf),
            "cT": np.ascontiguousarray(cb.T, dtype=bf),
            "Wq": wq, "Wkv": wkv, "Wout": wout,
            "maskb": np.ascontiguousarray(mb.T), "sel": sel,
        })
    return in_maps


def run(inputs: dict, trace: bool = False):
    if "nc" not in _CACHE:
        _CACHE["nc"] = _build()
    nc = _CACHE["nc"]
    in_maps = _prep_in_maps(**inputs)
    res = run_bass_kernel_spmd(nc, in_maps, core_ids=list(range(B)), trace=trace)
    out = np.stack([res.results[i]["out"] for i in range(B)]).astype(np.float32)
    out += np.asarray(inputs["bout"], dtype=np.float32)[None, None, :]
    return out, res


def kernel(**inputs) -> np.ndarray:
    out, _ = run(inputs, trace=False)
    return out


# revision 17
# speedup vs baseline: 1.1613x; 1.0053x over previous
"""Trainium2 Bass kernel for nn_Attention_81956565942967.

Cross-attention with key-length masking:
  B=8, N=1024, DIM=1024, HEADS=16, DIM_HEAD=64.

Sharding: pure data parallel — batch element b -> NeuronCore b. No
collectives. Host-side prep per shard: transpose x/context (so the
contraction dim lands on SBUF partitions) and cast the big operands to
bf16; compute the key-mask bias from lengths. bout is added host-side
(it is an epilogue broadcast add; exact in fp32).

Device algorithm (per core, T-layout: features on partitions):
  qT = Wq^T xT          kT = Wk^T cT          v = (cT)^T Wv   (natural)
  per head-pair hp, query tile i, per KEY TILE j:
    dotsT[j,i] two matmuls (sub=0/1 -> PE row groups 0/64, concurrent)
      into a [128, 2, 512] PSUM unit (double-buffered pool)
    one Exp ACT call per unit -> etall[:, j, :, :]   (bf16)
  per (hp, sub): po = [v_h | mask-diag]^T et_h  (K=128, M=80; psum rows
    64:80 carry the masked rowsum for head h at row 64+h)
  norm: reciprocal_approx_fast of the 16 rowsums, cast to bf16, K=16
    selector matmul broadcasts 1/rs across partitions; catT *= bcast.
  out = catT^T Wout  (bf16 output; bout added host-side)
Scheduling: qT/kT projections and early output-projection tiles are
interleaved into the attention loop as PE gap-filler; dots units are
double-buffered against the Exp stream so PE never waits on ACT.
"""

from contextlib import ExitStack

import ml_dtypes
import numpy as np

import concourse.bass as bass
from concourse import bacc
import concourse.mybir as mybir
import concourse.tile as tile
from concourse.bass_utils import run_bass_kernel_spmd

B, N, DIM = 8, 1024, 1024
HEADS, DIM_HEAD = 16, 64
INNER = HEADS * DIM_HEAD
SCALE = DIM_HEAD ** -0.5

P = 128
NT = N // P      # 8 partition tiles along n/j
KT = DIM // P    # 8 contraction tiles along dim/inner
FI = 512         # free-dim tile (PSUM bank)
NI = N // FI     # 2 query tiles
VW = 80  # v block per head: 64 dims + mask col at 64+h -> rowsum lands on psum row 64+h

BF = mybir.dt.bfloat16
F32 = mybir.dt.float32

_CACHE: dict = {}


def _build() -> bass.Bass:
    nc = bacc.Bacc("TRN2")

    xT_d = nc.dram_tensor("xT", [DIM, N], BF, kind="ExternalInput").ap()
    cT_d = nc.dram_tensor("cT", [DIM, N], BF, kind="ExternalInput").ap()
    wq_d = nc.dram_tensor("Wq", [DIM, INNER], BF, kind="ExternalInput").ap()
    wkv_d = nc.dram_tensor("Wkv", [DIM, 2 * INNER], BF, kind="ExternalInput").ap()
    wout_d = nc.dram_tensor("Wout", [INNER, DIM], BF, kind="ExternalInput").ap()
    mask_d = nc.dram_tensor("maskb", [P, NT], F32, kind="ExternalInput").ap()
    sel_d = nc.dram_tensor("sel", [80, (HEADS // 2) * P], BF, kind="ExternalInput").ap()
    out_d = nc.dram_tensor("out", [N, DIM], BF, kind="ExternalOutput").ap()

    with tile.TileContext(nc) as tc, ExitStack() as ctx:
        const_p = ctx.enter_context(tc.tile_pool(name="const", bufs=1))
        et_p = ctx.enter_context(tc.tile_pool(name="etp", bufs=2))
        stage_p = ctx.enter_context(tc.tile_pool(name="stage", bufs=3))
        acc_ps = ctx.enter_context(tc.tile_pool(name="accps", bufs=2, space="PSUM"))
        dots_ps = ctx.enter_context(tc.tile_pool(name="dotsps", bufs=2, space="PSUM"))
        po_ps = ctx.enter_context(tc.tile_pool(name="pops", bufs=2, space="PSUM"))

        # --- static SBUF tensors (declare before DMAs so order is explicit) ---
        xT_t = [const_p.tile([P, N], BF, tag=f"xslot{t}", name=f"xs{t}") for t in range(KT)]
        cT_t = [const_p.tile([P, N], BF, tag=f"cslot{t}", name=f"cs{t}") for t in range(KT)]
        wq_t = [const_p.tile([P, INNER], BF, tag=f"wq{t}", name=f"wq{t}") for t in range(KT)]
        wkv_t = [const_p.tile([P, 2 * INNER], BF, tag=f"wkv{t}", name=f"wkv{t}") for t in range(KT)]
        wout_t = [const_p.tile([P, DIM], BF, tag=f"cslot{t}", name=f"wo{t}") for t in range(KT)]
        qT_t = [const_p.tile([P, N], BF, tag=f"q{t}", name=f"qt{t}") for t in range(KT)]
        kT_t = [const_p.tile([P, N], BF, tag=f"k{t}", name=f"kt{t}") for t in range(KT)]
        v_t = [const_p.tile([P, HEADS * VW], BF, tag=f"v{t}", name=f"vt{t}") for t in range(NT)]
        catT_t = [const_p.tile([P, N], BF, tag=f"cat{t}", name=f"cat{t}") for t in range(KT)]

        mask_sb = const_p.tile([P, NT], F32, tag="mask")  # 1.0 valid / 0.0 masked
        sel_sb = const_p.tile([80, (HEADS // 2) * P], BF, tag="sel")
        rs_all = const_p.tile([80, N], F32, tag="rsall")
        rr_all = const_p.tile([80, N], F32, tag="rrall")
        rr_bf = const_p.tile([80, N], BF, tag="rrbf")

        # --- input DMAs, issued from three sequencers in parallel so
        # descriptor issue does not serialize the start:
        #   sync:   wq[0] + xT (q-projection chain)
        #   scalar: wkv k-col-0 slices + cT + mask (k-projection chain)
        #   gpsimd: wkv v-halves, remaining wkv k-cols, wq[1..], sel
        SPLIT = 4
        for s in range(SPLIT):
            sl = slice(s * (INNER // SPLIT), (s + 1) * (INNER // SPLIT))
            nc.sync.dma_start(out=wq_t[0][:, sl], in_=wq_d[0:P, sl])
        for s in range(SPLIT):
            sl = slice(s * (N // SPLIT), (s + 1) * (N // SPLIT))
            nc.sync.dma_start(out=xT_t[0][:, sl], in_=xT_d[0:P, sl])
        for t in range(1, KT):
            nc.sync.dma_start(out=xT_t[t], in_=xT_d[t * P:(t + 1) * P, :])
        for t in range(KT):  # just the wkv columns proj_k(0) contracts
            nc.scalar.dma_start(out=wkv_t[t][:, 0:P],
                                in_=wkv_d[t * P:(t + 1) * P, 0:P])
        for t in range(KT):
            nc.scalar.dma_start(out=cT_t[t], in_=cT_d[t * P:(t + 1) * P, :])
        nc.scalar.dma_start(out=mask_sb, in_=mask_d)
        for t in range(KT):
            nc.gpsimd.dma_start(out=wkv_t[t][:, INNER:2 * INNER],
                                in_=wkv_d[t * P:(t + 1) * P, INNER:2 * INNER])
        for t in range(KT):
            nc.gpsimd.dma_start(out=wkv_t[t][:, P:INNER],
                                in_=wkv_d[t * P:(t + 1) * P, P:INNER])
            if t > 0:
                nc.gpsimd.dma_start(out=wq_t[t], in_=wq_d[t * P:(t + 1) * P, :])
        nc.gpsimd.dma_start(out=sel_sb, in_=sel_d)
        for t in range(NT):
            nc.vector.memset(v_t[t], 0.0)
        nc.vector.memset(rs_all[64:80, :], 0.0)

        # --- projection helpers ---
        def proj_q(m, only_i=None):
            for i in range(NI) if only_i is None else [only_i]:
                ps = acc_ps.tile([P, FI], F32, tag="acc", name="ps")
                for k in range(KT):
                    nc.tensor.matmul(
                        ps,
                        wq_t[k][:, m * P:(m + 1) * P],
                        xT_t[k][:, i * FI:(i + 1) * FI],
                        start=(k == 0), stop=(k == KT - 1),
                    )
                nc.vector.tensor_copy(qT_t[m][:, i * FI:(i + 1) * FI], ps)

        def proj_k(m):
            for i in range(NI):
                ps = acc_ps.tile([P, FI], F32, tag="acc", name="ps")
                for k in range(KT):
                    nc.tensor.matmul(
                        ps,
                        wkv_t[k][:, m * P:(m + 1) * P],
                        cT_t[k][:, i * FI:(i + 1) * FI],
                        start=(k == 0), stop=(k == KT - 1),
                    )
                nc.vector.tensor_copy(kT_t[m][:, i * FI:(i + 1) * FI], ps)

        def proj_v(t, i2):
            # i2=0 computes v for heads 0..7, i2=1 for heads 8..15
            v3 = v_t[t].rearrange("p (h w) -> p h w", w=VW)
            ps = acc_ps.tile([P, FI], F32, tag="acc", name="ps")
            for k in range(KT):
                nc.tensor.matmul(
                    ps,
                    cT_t[k][:, t * P:(t + 1) * P],
                    wkv_t[k][:, INNER + i2 * FI:INNER + (i2 + 1) * FI],
                    start=(k == 0), stop=(k == KT - 1),
                )
            nc.vector.tensor_copy(
                v3[:, i2 * 8:(i2 + 1) * 8, 0:DIM_HEAD],
                ps.rearrange("p (h d) -> p h d", d=DIM_HEAD),
            )

        def v_diag(t):
            # mask col of head h sits at flat position h*VW + 64 + h = 64 + 81*h
            diag = bass.AP(tensor=v_t[t].tensor, offset=v_t[t].offset + DIM_HEAD,
                           ap=[list(v_t[t].ap[0]), [VW + 1, HEADS]])
            nc.vector.tensor_scalar_mul(
                diag, mask_sb[:, t:t + 1].to_broadcast([P, HEADS]), 1.0)

        def final_group(t, i, tail=False):
            pf = acc_ps.tile([P, FI], F32, tag="acc", name="pf")
            for k in range(KT):
                nc.tensor.matmul(
                    pf,
                    catT_t[k][:, t * P:(t + 1) * P],
                    wout_t[k][:, i * FI:(i + 1) * FI],
                    start=(k == 0), stop=(k == KT - 1),
                )
            ot = stage_p.tile([P, FI], BF, tag="ot", name="ot")
            if tail:
                nc.scalar.copy(ot, pf)  # ACT engine is idle at the kernel tail
            else:
                nc.vector.tensor_copy(ot, pf)
            dst = out_d[t * P:(t + 1) * P, i * FI:(i + 1) * FI]
            if tail:
                # stripe the tail output DMAs across queues to shrink drain
                for s in range(2):
                    nc.sync.dma_start(
                        out=dst[:, s * (FI // 2):(s + 1) * (FI // 2)],
                        in_=ot[:, s * (FI // 2):(s + 1) * (FI // 2)],
                    )
            else:
                nc.sync.dma_start(out=dst, in_=ot)

        def dots_exp(i, hp):
            isl = slice(i * FI, (i + 1) * FI)
            pt = hp
            et = et_p.tile([P, NT, 2, FI], BF, tag="et", name="et")
            # per key-tile unit: paired dots (heads 2hp/2hp+1 at PE row
            # groups 0/64 -> concurrent) + one Exp call; double-buffered
            for j in range(NT):
                dps = dots_ps.tile([P, 2, FI], F32, tag="dots", name="dps")
                for sub in range(2):
                    off = sub * DIM_HEAD
                    nc.tensor.matmul(
                        dps[:, sub, :],
                        kT_t[pt][off:off + DIM_HEAD, j * P:(j + 1) * P],
                        qT_t[pt][off:off + DIM_HEAD, isl],
                        start=True, stop=True,
                    )
                nc.scalar.activation(
                    et[:, j, :, :], dps,
                    mybir.ActivationFunctionType.Exp, scale=SCALE,
                )
            return et

        def po_part(i, hp, et):
            # NOTE: must be emitted after proj_v — the po read of the v
            # mask-diag (strided AP) is not reliably dependency-tracked.
            isl = slice(i * FI, (i + 1) * FI)
            pt = hp
            for sub in range(2):
                h = 2 * hp + sub
                off = sub * DIM_HEAD
                po = po_ps.tile([VW, FI], F32, tag="po", name="po")
                for j in range(NT):
                    nc.tensor.matmul(
                        po,
                        v_t[j][:, h * VW:(h + 1) * VW],
                        et[:, j, sub, :],
                        start=(j == 0), stop=(j == NT - 1),
                    )
                cslice = catT_t[pt][off:off + DIM_HEAD, isl]
                nc.vector.tensor_copy(cslice, po[0:DIM_HEAD, :])
                nc.vector.tensor_tensor(
                    rs_all[64:80, isl], rs_all[64:80, isl], po[64:80, :],
                    mybir.AluOpType.add)

        def attention(i, hp):
            et = dots_exp(i, hp)
            po_part(i, hp, et)

        def norm(i):
            # 1/rs via exp(-ln(rs)) on ACT: ln and exp share the
            # natural_log_exp table set, and exp writes bf16 directly.
            isl = slice(i * FI, (i + 1) * FI)
            nc.scalar.activation(rr_all[64:80, isl], rs_all[64:80, isl],
                                 mybir.ActivationFunctionType.Ln)
            nc.scalar.activation(rr_bf[64:80, isl], rr_all[64:80, isl],
                                 mybir.ActivationFunctionType.Exp, scale=-1.0)
            for pt in range(HEADS // 2):
                pb = acc_ps.tile([P, FI], F32, tag="acc", name="pb")
                nc.tensor.matmul(
                    pb,
                    sel_sb[64:80, pt * P:(pt + 1) * P],
                    rr_bf[64:80, isl],
                    start=True, stop=True,
                )
                nc.vector.tensor_tensor(
                    catT_t[pt][:, isl], catT_t[pt][:, isl], pb,
                    mybir.AluOpType.mult,
                )

        # --- schedule: the dots->exp stream starts as early as possible
        # and runs with po lagging one pair behind (et pool bufs=2);
        # projections are PE gap-filler. po_part must be EMITTED after the
        # proj_v/v_diag writes it reads (strided APs are not reliably
        # dependency-tracked), hence the explicit lag structure. ---
        proj_q(0)
        proj_k(0)
        proj_q(1, only_i=0)
        proj_k(1)
        ets = {}
        ets[0] = dots_exp(0, 0)
        for t in range(NT):
            proj_v(t, 0)          # heads 0..7 (pairs 0..3)
        for t in range(NT):
            v_diag(t)
        proj_q(2, only_i=0)
        proj_k(2)
        ets[1] = dots_exp(0, 1)
        po_part(0, 0, ets[0])
        for t in range(NT // 2):
            proj_v(t, 1)          # heads 8..15 (pairs 4..7)
        ets[2] = dots_exp(0, 2)
        po_part(0, 1, ets[1])
        for t in range(NT // 2, NT):
            proj_v(t, 1)
        proj_q(3, only_i=0)
        proj_k(3)
        for hp in range(3, HEADS // 2):
            ets[hp] = dots_exp(0, hp)
            po_part(0, hp - 1, ets[hp - 1])
            if hp + 1 < KT:
                proj_q(hp + 1, only_i=0)
                proj_k(hp + 1)
        po_part(0, 7, ets[7])
        for t in range(KT):
            nc.sync.dma_start(out=wout_t[t], in_=wout_d[t * P:(t + 1) * P, :])

        # phase 1: keep the ACT stream rolling; qT i=1 halves projected
        # one pair ahead; finals for query tiles 0..3 are the gap-filler
        ets[8] = dots_exp(1, 0)
        proj_q(1, only_i=1)
        norm(0)
        fin = [(t, i) for t in range(4) for i in range(NI)]
        for hp in range(1, HEADS // 2):
            ets[8 + hp] = dots_exp(1, hp)
            po_part(1, hp - 1, ets[7 + hp])
            if hp + 1 < KT:
                proj_q(hp + 1, only_i=1)
            final_group(*fin[hp - 1])
        po_part(1, 7, ets[15])
        final_group(*fin[7])
        norm(1)
        for t in range(4, NT):
            for i in range(NI):
                final_group(t, i, tail=True)

    nc.finalize()
    return nc


def _prep_in_maps(x, context, lengths, Wq, Wkv, Wout, bout):
    bf = ml_dtypes.bfloat16
    wq = np.ascontiguousarray(Wq, dtype=bf)
    wkv = np.ascontiguousarray(Wkv, dtype=bf)
    wout = np.ascontiguousarray(Wout, dtype=bf)
    jj = np.arange(N).reshape(NT, P)  # [j_tile, partition]
    sel = np.zeros((80, (HEADS // 2) * P), dtype=bf)
    for pt in range(HEADS // 2):
        sel[64 + 2 * pt, pt * P:pt * P + DIM_HEAD] = 1.0
        sel[65 + 2 * pt, pt * P + DIM_HEAD:(pt + 1) * P] = 1.0
    in_maps = []
    context = np.asarray(context)
    for b in range(B):
        mb = np.where(jj < int(lengths[b]), 1.0, 0.0).astype(np.float32)
        cb = context[b].copy()
        cb[int(lengths[b]):] = 0.0
        in_maps.append({
            "xT": np.ascontiguousarray(np.asarray(x[b]).T, dtype=bf),
            "cT": np.ascontiguousarray(cb.T, dtype=bf),
            "Wq": wq, "Wkv": wkv, "Wout": wout,
            "maskb": np.ascontiguousarray(mb.T), "sel": sel,
        })
    return in_maps


def run(inputs: dict, trace: bool = False):
    if "nc" not in _CACHE:
        _CACHE["nc"] = _build()
    nc = _CACHE["nc"]
    in_maps = _prep_in_maps(**inputs)
    res = run_bass_kernel_spmd(nc, in_maps, core_ids=list(range(B)), trace=trace)
    out = np.stack([res.results[i]["out"] for i in range(B)]).astype(np.float32)
    out += np.asarray(inputs["bout"], dtype=np.float32)[None, None, :]
    return out, res


def kernel(**inputs) -> np.ndarray:
    out, _ = run(inputs, trace=False)
    return out
